# revision 9
# baseline (speedup 1.0000x reference)
"""ASTRF kernel for Trainium2 (8 NeuronCores, axon).

Math: out[b,o,t] = sum_{i,w} weight[o,i,w] * xs[b,i,t-w] + bias[o]
where xs[b,i,src[b,s]] = x[b,i,s] (scatter of events to onsets).

Device scheme (per core = 2 jobs of (batch, 128-block group)):
  time t = 64*m + q, q = 8c + q'.  Out[m, (q,o)-tile c] =
      sum_{(u,i)} Xblk[m][(u,i)] * W0[c-a] + Xblk[m-1][(u,i)] * W1[a-c]
  K = (u,i) = 1024 -> 8 chunks of 128 partitions; M = 128 blocks; N = 512
  (n = q'*64 + o).  9 accumulating matmuls per (job, c) into one PSUM bank.
  Weights live in one shifted tensor Wsh[(u',i), zz', o] = weight[o,i,zz'-u']
  (zero-padded), so tile k's rhs is the window zz' in [8k, 8k+8): chunk k
  serves W0[d=k] and W1[e=8-k].  DVE drains PSUM into a staging tile with
  fused bias add; per-c 256KB contiguous DMAs to HBM.  Host does the (cheap)
  input scatter into block layout and the final unshuffle.
"""

import sys

for _p in ("/opt/trn_rl_repo", "/root/.axon_site/_ro/trn_rl_repo"):
    if _p not in sys.path:
        sys.path.insert(0, _p)

import numpy as np

B, I, S = 4, 16, 4096
O, W = 64, 64
T = 32768
NBLK = T // 64            # 512 blocks per batch
N_CORES = 8

_prog_cache = {}


def _mats_for(c):
    """(chunk a, wsh window k, lhsT col offset) in ascending-k (DMA) order."""
    mats = [(c, 0, 1)]                                   # W0[0]
    mats += [(c - k, k, 1) for k in range(1, c + 1)]     # W0[k]
    mats += [(c + 8 - k, k, 0) for k in range(c + 1, 9)]  # W1[8-k]
    return mats


def _build_program():
    if "nc" in _prog_cache:
        return _prog_cache["nc"]
    import concourse.bacc as bacc
    import concourse.mybir as mybir
    import concourse.tile as tile

    f32 = mybir.dt.float32
    f32r = mybir.dt.float32r
    nc = bacc.Bacc("TRN2", target_bir_lowering=False, debug=False, num_devices=N_CORES)

    xin = nc.dram_tensor("xin", [128, 2, 8, 129], f32r, kind="ExternalInput")
    wshd = nc.dram_tensor("wshd", [128, 72, 64], f32r, kind="ExternalInput")
    biasrep = nc.dram_tensor("biasrep", [128, 512], f32, kind="ExternalInput")
    out = nc.dram_tensor("out", [2, 8, 128, 512], f32, kind="ExternalOutput")

    with tile.TileContext(nc) as tc:
        with (
            tc.tile_pool(name="const", bufs=1) as cpool,
            tc.tile_pool(name="stage", bufs=2) as spool,
            tc.tile_pool(name="psum", bufs=1, space="PSUM") as ppool,
        ):
            x_sb = cpool.tile([128, 2, 8, 129], f32r, tag="x")
            wsh_sb = cpool.tile([128, 72, 64], f32r, tag="wsh")
            b_sb = cpool.tile([128, 512], f32, tag="bias")
            for a in range(8):
                nc.sync.dma_start(out=x_sb[:, 0, a, :], in_=xin[:, 0, a, :])
            for k in range(9):
                nc.sync.dma_start(
                    out=wsh_sb[:, 8 * k : 8 * k + 8, :],
                    in_=wshd[:, 8 * k : 8 * k + 8, :],
                )
            nc.sync.dma_start(out=b_sb[:], in_=biasrep[:])
            for a in range(8):
                nc.sync.dma_start(out=x_sb[:, 1, a, :], in_=xin[:, 1, a, :])

            for j in range(2):
                stage = spool.tile([128, 8, 512], f32, tag="stage")
                for c in range(8):
                    ps = ppool.tile(
                        [128, 512], f32, tag=f"ps{c}", name=f"ps{j}_{c}"
                    )
                    for idx, (a, k, col0) in enumerate(_mats_for(c)):
                        nc.tensor.matmul(
                            out=ps[:],
                            lhsT=x_sb[:, j, a, col0 : col0 + 128],
                            rhs=wsh_sb[:, 8 * k : 8 * k + 8, :],
                            start=(idx == 0),
                            stop=(idx == 8),
                        )
                    nc.vector.tensor_add(out=stage[:, c, :], in0=ps[:], in1=b_sb[:])
                    nc.sync.dma_start(out=out[j, c], in_=stage[:, c, :])

    nc.compile()
    _prog_cache["nc"] = nc
    return nc


def _host_pack(x, weight, bias, sourceIdx):
    """Build per-core device inputs from full inputs."""
    # --- scatter x into blocked layout xs6[b, a, u', i, col=m+1] ---
    xs6 = np.zeros((B, 8, 8, I, NBLK + 1), np.float32)
    src = np.asarray(sourceIdx, np.int64)
    for b in range(B):
        t = src[b]
        m = t >> 6
        u = t & 63
        xs6[b, u >> 3, u & 7, :, m + 1] = np.asarray(x[b], np.float32).T
    # per-core X: [p=u'*16+i, j, a, col']
    x_cores = []
    for core in range(N_CORES):
        b, h = divmod(core, 2)
        tmp = xs6[b].transpose(1, 2, 0, 3).reshape(128, 8, NBLK + 1)
        arr = np.empty((128, 2, 8, 129), np.float32)
        for j in range(2):
            g = 2 * h + j
            arr[:, j, :, :] = tmp[:, :, 128 * g : 128 * g + 129]
        x_cores.append(np.ascontiguousarray(arr))

    # --- shifted weights Wsh[p=(u',i), zz', o] = weight[o,i,zz'-u'] ---
    wgt = np.asarray(weight, np.float32)  # (O, I, W)
    zz = np.arange(72)
    up = np.arange(8)
    idx = zz[None, :] - up[:, None]              # (8 u', 72 zz')
    valid = (idx >= 0) & (idx < W)
    g = wgt[:, :, np.clip(idx, 0, W - 1)] * valid[None, None]  # (O, I, 8, 72)
    wsh_host = np.ascontiguousarray(
        g.transpose(2, 1, 3, 0).reshape(128, 72, 64), dtype=np.float32
    )
    # bias replicated, n = q'*64 + o
    bias_rep = np.tile(
        np.tile(np.asarray(bias, np.float32), 8)[None, :], (128, 1)
    ).astype(np.float32)
    return x_cores, wsh_host, bias_rep


def kernel(x, weight, bias, sourceIdx, nRealLen, _trace=False, _trace_out=None):
    from concourse import bass_utils

    nRealLen = int(nRealLen)
    assert nRealLen == T, f"kernel hardcoded for nRealLen={T}, got {nRealLen}"
    x_cores, wsh_host, bias_rep = _host_pack(x, weight, bias, sourceIdx)
    nc = _build_program()
    in_maps = [
        {"xin": x_cores[c], "wshd": wsh_host, "biasrep": bias_rep}
        for c in range(N_CORES)
    ]
    res = bass_utils.run_bass_kernel_spmd(
        nc,
        in_maps,
        core_ids=list(range(N_CORES)),
        trace=_trace,
        trace_cores=list(range(N_CORES)) if _trace else None,
    )
    if _trace_out is not None:
        _trace_out.append(res)
    out_full = np.empty((B, O, T), np.float32)
    for core in range(N_CORES):
        b, h = divmod(core, 2)
        r = res.results[core]["out"]  # (2, 8, 128, 512)
        r5 = r.reshape(2, 8, 128, 8, 64)  # [j, c, m, q', o]
        for j in range(2):
            g = 2 * h + j
            seg = r5[j].transpose(3, 1, 0, 2).reshape(64, 8192)
            out_full[b, :, g * 8192 : (g + 1) * 8192] = seg
    return out_full
